# revision 8
# baseline (speedup 1.0000x reference)
"""ACDA kernel for 8 TRN2 NeuronCores — pure data-parallel over batch (B=8).

Math (per image):
  out[o,p] = sum_{a,k} relu((w[o,a,k,:]@x[:,p] + b[o,a,k]) * att[a,p]) * x[o, p+d(k)]
using relu(z)*att = relu(z*att) (att = exp(-d^2) > 0), so the per-pixel atom
scaling folds into the matmul rhs: xA[a] = x * att[a], bias via an extra
contraction row holding att[a] with weights b.

Device pipeline per 512-pixel chunk (8 image rows):
  TensorE: 4 atoms x 5 M-tiles matmul (K=65) -> PSUM;  20 reduce-matmuls with a
           block-ones [128,64] stationary accumulate out[64,512] in PSUM.
  ScalarE: relu PSUM->SBUF bf16.
  VectorE: xA builds (x*att) and prod = relu_filt * x_shifted (taps via
           column-shifted APs into a zero-padded 66x66 image layout).
"""

import numpy as np
import ml_dtypes

BF = ml_dtypes.bfloat16

B, C, O, H, W = 8, 64, 64, 64, 64
A, KK = 4, 9
HW = H * W  # 4096
PWID = 66 * 66 + 68  # padded 66x66 image + slack so rearrange slices stay in-bounds
NCHUNK = 8
CH = 512  # pixels per chunk (8 image rows)

# tap order: chosen so tile j rows [0:64]=tap 2j, [64:128]=tap 2j+1 pair up with
# column-stagger tricks: x2p hi-half = pad-image shifted +1 col, x3p hi = +66.
MY_TAPS = [(-1, -1), (-1, 0), (0, -1), (0, 0), (1, -1), (1, 0), (-1, 1), (0, 1), (1, 1)]
# per j-tile: (which padded tile, base column delta) for the even tap of the pair
TAP_SRC = [("x2p", -67), ("x2p", -1), ("x2p", 65), ("x3p", -65), ("x2p", 67)]

_CACHE = {}


def _host_arrays(w_gen, b_gen, pos_enc):
    w4 = w_gen.reshape(O, A, KK, C)
    b4 = b_gen.reshape(O, A, KK)
    waug = np.zeros((A, 65, 640), np.float32)
    for mk, (dh, dw) in enumerate(MY_TAPS):
        kref = (dh + 1) * 3 + (dw + 1)
        for a in range(A):
            waug[a, 0:64, mk * 64:(mk + 1) * 64] = w4[:, a, kref, :].T
            waug[a, 64, mk * 64:(mk + 1) * 64] = b4[:, a, kref]

    ys = np.linspace(-1.0, 1.0, H, dtype=np.float32)
    xs = np.linspace(-1.0, 1.0, W, dtype=np.float32)
    att = np.exp(-((xs[None, None, :] - pos_enc[:, 0, None, None]) ** 2
                   + (ys[None, :, None] - pos_enc[:, 1, None, None]) ** 2))
    att = att.reshape(A, HW).astype(np.float32)
    attb = np.broadcast_to(att[:, None, :], (A, 65, HW)).astype(BF).copy()

    ones64 = np.zeros((128, 64), np.float32)
    r = np.arange(128)
    ones64[r, r % 64] = 1.0
    return waug.astype(BF), attb, ones64.astype(BF)


def _build_graph(dbg=False):
    import concourse.bass as bass
    import concourse.bacc as bacc
    import concourse.mybir as mybir
    from concourse import tile

    f32 = mybir.dt.float32
    bf16 = mybir.dt.bfloat16
    Relu = mybir.ActivationFunctionType.Relu
    Copy = mybir.ActivationFunctionType.Copy

    nc = bacc.Bacc(None, target_bir_lowering=False, debug=False)

    xbf_d = nc.declare_dram_parameter("xbf", [C, HW], bf16, isOutput=False)
    attb_d = nc.declare_dram_parameter("attb", [A, 65, HW], bf16, isOutput=False)
    waug_d = nc.declare_dram_parameter("waug", [A, 65, 640], bf16, isOutput=False)
    ones_d = nc.declare_dram_parameter("ones", [128, 64], bf16, isOutput=False)
    out_d = nc.declare_dram_parameter("out", [O, HW], f32, isOutput=True)
    if dbg:
        dbg_x2p = nc.declare_dram_parameter("dbg_x2p", [128, PWID], bf16, isOutput=True)
        dbg_x3p = nc.declare_dram_parameter("dbg_x3p", [128, PWID], bf16, isOutput=True)
        dbg_xa0 = nc.declare_dram_parameter("dbg_xa0", [65, HW], bf16, isOutput=True)
        dbg_fs = nc.declare_dram_parameter("dbg_fs", [128, CH], bf16, isOutput=True)
        dbg_pr = nc.declare_dram_parameter("dbg_pr", [128, CH], bf16, isOutput=True)

    with tile.TileContext(nc) as tc:
        with (
            tc.tile_pool(name="persist", bufs=1) as pp,
            tc.tile_pool(name="fs", bufs=3) as fsp,
            tc.tile_pool(name="pr", bufs=3) as prp,
            tc.tile_pool(name="osb", bufs=2) as osp,
            tc.tile_pool(name="psf", bufs=4, space=bass.MemorySpace.PSUM) as psf,
            tc.tile_pool(name="pso", bufs=2, space=bass.MemorySpace.PSUM) as pso,
        ):
            waug_sb = []
            for a in range(A):
                w_t = pp.tile([65, 640], bf16, tag=f"waug{a}", name=f"waug{a}")
                nc.sync.dma_start(w_t[:], waug_d[a])
                waug_sb.append(w_t)
            ones_sb = pp.tile([128, 64], bf16, tag="ones")
            nc.sync.dma_start(ones_sb[:], ones_d[:])

            xsb = pp.tile([C, HW], bf16, tag="xsb")
            nc.sync.dma_start(xsb[:], xbf_d[:])

            attb_sb, xa_sb = [], []
            for a in range(A):
                at_t = pp.tile([65, HW], bf16, tag=f"attb{a}", name=f"attb{a}")
                nc.sync.dma_start(at_t[:], attb_d[a])
                attb_sb.append(at_t)
                xa_sb.append(pp.tile([65, HW], bf16, tag=f"xa{a}", name=f"xa{a}"))

            # padded 66x66 image copies; hi half = +1 col / +66 col stagger
            x2p = pp.tile([128, PWID], bf16, tag="x2p")
            x3p = pp.tile([128, PWID], bf16, tag="x3p")
            nc.vector.memset(x2p[:], 0.0)
            nc.vector.memset(x3p[:], 0.0)

            xim = xsb[:].rearrange("p (h w) -> p h w", w=W)
            xdrm = xbf_d[:].rearrange("p (h w) -> p h w", w=W)
            for t in (x2p, x3p):
                dst = t[0:64, 66:66 + 64 * 66].rearrange("p (r c) -> p r c", c=66)[:, :, 1:65]
                nc.vector.tensor_copy(dst, xim)
            # hi halves obey x2p_hi[i] = x_pad[i+1], x3p_hi[i] = x_pad[i+66]:
            # d2: abs block B in 1..64, cols 0:64 <- x row B-1
            d2 = x2p[64:128, 66:66 + 64 * 66].rearrange("p (r c) -> p r c", c=66)[:, :, 0:64]
            nc.sync.dma_start(d2, xdrm[:, :, :])
            # d3: abs block B in 0..63, cols 1:65 <- x row B
            d3 = x3p[64:128, 0:64 * 66].rearrange("p (r c) -> p r c", c=66)[:, :, 1:65]
            nc.sync.dma_start(d3, xdrm[:, :, :])

            for a in range(A):
                nc.vector.tensor_copy(xa_sb[a][64:65, :], attb_sb[a][64:65, :])

            src_map = {"x2p": x2p, "x3p": x3p}
            for ci in range(NCHUNK):
                c0 = CH * ci
                h0 = 8 * ci
                ops = pso.tile([64, CH], f32, tag="ops")
                for a in range(A):
                    nc.vector.tensor_mul(
                        xa_sb[a][0:64, c0:c0 + CH],
                        xsb[:, c0:c0 + CH],
                        attb_sb[a][0:64, c0:c0 + CH],
                    )
                    for j in range(5):
                        ft = psf.tile([128, CH], f32, tag="ft")
                        nc.tensor.matmul(
                            ft[:],
                            waug_sb[a][:, j * 128:(j + 1) * 128],
                            xa_sb[a][:, c0:c0 + CH],
                            start=True, stop=True,
                        )
                        fs = fsp.tile([128, CH], bf16, tag="fs")
                        nc.scalar.activation(fs[:], ft[:], Relu)
                        pr = prp.tile([128, CH], bf16, tag="pr")
                        sname, delta = TAP_SRC[j]
                        base = (h0 + 1) * 66 + 1 + delta
                        in1 = src_map[sname][:, base:base + 8 * 66].rearrange(
                            "p (r c) -> p r c", c=66)[:, :, 0:64]
                        nc.vector.tensor_mul(
                            pr[:].rearrange("p (r c) -> p r c", c=64),
                            fs[:].rearrange("p (r c) -> p r c", c=64),
                            in1,
                        )
                        nc.tensor.matmul(
                            ops[:], ones_sb[:], pr[:],
                            start=(a == 0 and j == 0),
                            stop=(a == 3 and j == 4),
                            skip_group_check=True,
                        )
                        if dbg and ci == 0 and a == 0 and j == 1:
                            nc.sync.dma_start(dbg_fs[:], fs[:])
                            nc.sync.dma_start(dbg_pr[:], pr[:])
                osb = osp.tile([64, CH], f32, tag="osb")
                nc.scalar.activation(osb[:], ops[:], Copy)
                nc.sync.dma_start(out_d[:, c0:c0 + CH], osb[:])

            if dbg:
                nc.sync.dma_start(dbg_x2p[:], x2p[:])
                nc.sync.dma_start(dbg_x3p[:], x3p[:])
                nc.sync.dma_start(dbg_xa0[:], xa_sb[0][:])

    nc.compile()
    return nc


def _in_maps(x, w_gen, b_gen, pos_enc):
    waug, attb, ones64 = _host_arrays(
        np.asarray(w_gen, np.float32), np.asarray(b_gen, np.float32),
        np.asarray(pos_enc, np.float32))
    x = np.asarray(x, np.float32)
    return [{
        "xbf": x[i].reshape(C, HW).astype(BF),
        "attb": attb,
        "waug": waug,
        "ones": ones64,
    } for i in range(B)]


def kernel(x, w_gen, b_gen, pos_enc):
    from concourse.bass_utils import run_bass_kernel_spmd

    if "nc" not in _CACHE:
        _CACHE["nc"] = _build_graph()
    nc = _CACHE["nc"]
    in_maps = _in_maps(x, w_gen, b_gen, pos_enc)
    res = run_bass_kernel_spmd(nc, in_maps, core_ids=list(range(B)), trace=False)
    _CACHE["last"] = res
    out = np.stack([
        np.asarray(res.results[i]["out"], dtype=np.float32).reshape(O, H, W)
        for i in range(B)
    ])
    return out


def bench(x, w_gen, b_gen, pos_enc, iters=20):
    """Wall-clock the sharded NEFF execution with device-resident inputs.

    Mirrors bass2jax.run_bass_via_pjrt's plumbing but keeps inputs on device
    and times repeated executions (min over iters). NTFF profiling is not
    available in this container, so this is the timing signal.
    """
    import time
    import jax
    import concourse.mybir as mybir
    from concourse import bass2jax
    from jax.sharding import Mesh, PartitionSpec
    from jax.experimental.shard_map import shard_map

    bass2jax.install_neuronx_cc_hook()
    if "nc" not in _CACHE:
        _CACHE["nc"] = _build_graph()
    nc = _CACHE["nc"]
    in_maps = _in_maps(x, w_gen, b_gen, pos_enc)

    partition_name = nc.partition_id_tensor.name if nc.partition_id_tensor else None
    in_names, out_names, out_avals, zero_outs = [], [], [], []
    for alloc in nc.m.functions[0].allocations:
        if not isinstance(alloc, mybir.MemoryLocationSet):
            continue
        name = alloc.memorylocations[0].name
        if alloc.kind == "ExternalInput":
            if name != partition_name:
                in_names.append(name)
        elif alloc.kind == "ExternalOutput":
            shape = tuple(alloc.tensor_shape)
            dtype = mybir.dt.np(alloc.dtype)
            out_names.append(name)
            out_avals.append(jax.core.ShapedArray(shape, dtype))
            zero_outs.append(np.zeros(shape, dtype))
    n_params = len(in_names)
    all_names = in_names + out_names
    if partition_name is not None:
        all_names = all_names + [partition_name]

    def _body(*args):
        operands = list(args)
        if partition_name is not None:
            operands.append(bass2jax.partition_id_tensor())
        outs = bass2jax._bass_exec_p.bind(
            *operands,
            out_avals=tuple(out_avals),
            in_names=tuple(all_names),
            out_names=tuple(out_names),
            lowering_input_output_aliases=(),
            sim_require_finite=True,
            sim_require_nnan=True,
            nc=nc,
        )
        return tuple(outs)

    devices = jax.devices()[:B]
    mesh = Mesh(np.asarray(devices), ("core",))
    in_specs = (PartitionSpec("core"),) * (n_params + len(out_names))
    out_specs = (PartitionSpec("core"),) * len(out_names)
    fn = jax.jit(shard_map(_body, mesh=mesh, in_specs=in_specs,
                           out_specs=out_specs, check_rep=False),
                 keep_unused=True)
    concat_in = [np.concatenate([np.asarray(in_maps[c][n]) for c in range(B)], axis=0)
                 for n in in_names]
    concat_zeros = [np.zeros((B * z.shape[0], *z.shape[1:]), z.dtype) for z in zero_outs]
    from jax.sharding import NamedSharding
    sh = NamedSharding(mesh, PartitionSpec("core"))
    dev_in = [jax.device_put(a, sh) for a in concat_in + concat_zeros]
    out = fn(*dev_in)
    jax.block_until_ready(out)
    times = []
    for _ in range(iters):
        t0 = time.perf_counter()
        out = fn(*dev_in)
        jax.block_until_ready(out)
        times.append(time.perf_counter() - t0)
    return min(times), sorted(times)[len(times) // 2]
